# revision 1
# baseline (speedup 1.0000x reference)
"""Trainium2 Bass kernel for nn_BaseAttention (B=2,S=2048,D=1024,H=16,K=64).

Sharding: 8 cores = (batch b in {0,1}) x (query-block qb in {0..3}, 512 rows).
Each core computes K/V projections for the FULL sequence of its batch
(redundant across the 4 cores sharing a batch -- avoids any collective),
attention + output projection + residual + LayerNorm for its 512 query rows.
Host concatenates the 8 [512, 1024] output blocks.

Layouts (per core):
  xT  [D, S]   : x[b] transposed (host-side) -> contraction dim d on partitions
  QT/KT [hk,s] : produced transposed, feeds scoresT = K^T(64) x Q^T directly
  scoresT [s,q]: softmax denominator obtained via an all-ones 65th column
                 appended to V (ctx matmul also emits sum_s exp = denom row)
  ctxT [hk, q] : feeds output projection with wo [hk, d] in natural layout
All big matmuls run in float32r (fp22 truncated fp32, full PE rate at N>=256).
"""

import sys
import numpy as np

B, S, D, H, KD = 2, 2048, 1024, 16, 64
SB = S // 4
HK = H * KD
P = 128

if "/opt/trn_rl_repo" not in sys.path:
    sys.path.insert(0, "/opt/trn_rl_repo")

_cache = {}


def _build(D_, S_, SB_, H_):
    import concourse.bass as bass
    import concourse.mybir as mybir
    from concourse.tile import TileContext

    dt = mybir.dt
    f32, f32r = dt.float32, dt.float32r
    AF = mybir.ActivationFunctionType
    OP = mybir.AluOpType
    AX = mybir.AxisListType.X

    HK_ = H_ * KD
    DC = D_ // P              # d-chunks of 128
    NSC = S_ // P             # s-chunks of 128
    S5 = min(512, S_)
    NS5 = S_ // S5            # s-chunks of 512 (moving dim)
    QN = SB_                  # scores moving width (<=512)
    NQC = SB_ // P
    D5 = min(512, D_)
    ND5 = D_ // D5
    NG = H_ // 4 if H_ >= 8 else 2    # head groups (SBUF residency)
    GH = H_ // NG
    GW = GH * KD              # group width in hk (<=512)
    NKT = GW // P             # KT tiles per group
    HC = HK_ // P
    assert QN <= 512 and GW <= 512

    nc = bass.Bass()
    xT = nc.declare_dram_parameter("xT", [D_, S_], f32r, isOutput=False)
    xqT = nc.declare_dram_parameter("xqT", [D_, SB_], f32r, isOutput=False)
    xq = nc.declare_dram_parameter("xq", [SB_, D_], f32, isOutput=False)
    wq = nc.declare_dram_parameter("wq", [D_, HK_], f32r, isOutput=False)
    wk = nc.declare_dram_parameter("wk", [D_, HK_], f32r, isOutput=False)
    wv = nc.declare_dram_parameter("wv", [D_, HK_], f32r, isOutput=False)
    wo = nc.declare_dram_parameter("wo", [HK_, D_], f32r, isOutput=False)
    bqp = nc.declare_dram_parameter("bqT", [P, HC], f32, isOutput=False)
    bkp = nc.declare_dram_parameter("bkT", [P, HC], f32, isOutput=False)
    bvp = nc.declare_dram_parameter("bv_row", [1, HK_], f32r, isOutput=False)
    bop = nc.declare_dram_parameter("bo_row", [1, D_], f32, isOutput=False)
    gmp = nc.declare_dram_parameter("gamma_row", [1, D_], f32, isOutput=False)
    btp = nc.declare_dram_parameter("beta_row", [1, D_], f32, isOutput=False)
    out = nc.declare_dram_parameter("out", [SB_, D_], f32, isOutput=True)

    with TileContext(nc) as tc:
        with tc.tile_pool(name="const", bufs=1) as cpool, \
             tc.tile_pool(name="ctxn", bufs=H_) as cnp:

            ones = cpool.tile([128, P], f32, tag="ones")
            nc.vector.memset(ones[:], 1.0)
            ones_r32 = cpool.tile([1, P], f32, tag="ones_r")
            nc.vector.memset(ones_r32[:], 1.0)
            ones_r = ones_r32[:].bitcast(f32r)
            eps_t = cpool.tile([P, 1], f32, tag="eps")
            nc.vector.memset(eps_t[:], 1e-6)
            # DMA order below is load-bearing: HW DMA queues are assigned
            # round-robin in scheduled order, and blocks of 8 tile-loads are
            # kept queue-aligned so each matmul's inputs share one queue
            # (walrus allows only one sync-wait on f32r matmuls).
            bq_sb = cpool.tile([P, HC], f32, tag="bq")
            nc.sync.dma_start(out=bq_sb[:], in_=bqp[:])
            bk_sb = cpool.tile([P, HC], f32, tag="bk")
            nc.sync.dma_start(out=bk_sb[:], in_=bkp[:])
            bv_sb = cpool.tile([1, HK_], f32r, tag="bv")
            nc.sync.dma_start(out=bv_sb[:], in_=bvp[:])

            # x^T resident for all projections (released before out-proj)
            xtq_scope = tc.tile_pool(name="xT", bufs=DC)
            xtp = xtq_scope.__enter__()
            qt_scope = tc.tile_pool(name="QT", bufs=HC)
            qtp = qt_scope.__enter__()
            xt_sb = []
            for dc in range(DC):
                t = xtp.tile([P, S_], f32r, tag="xt")
                nc.sync.dma_start(out=t[:], in_=xT[dc * P:(dc + 1) * P, :])
                xt_sb.append(t)

            # ---- Q^T [hk, q] = wq^T x xqT, + bias, x 1/sqrt(K) ----
            qt_sb = []
            with tc.tile_pool(name="wq", bufs=DC) as wqp, \
                 tc.tile_pool(name="xqT", bufs=DC) as xqp, \
                 tc.tile_pool(name="qps", bufs=2, space="PSUM") as qps:
                wq_sb, xq_sb = [], []
                for dc in range(DC):
                    t = wqp.tile([P, HK_], f32r, tag="wq")
                    nc.sync.dma_start(out=t[:], in_=wq[dc * P:(dc + 1) * P, :])
                    wq_sb.append(t)
                for dc in range(DC):
                    t = xqp.tile([P, SB_], f32r, tag="xq")
                    nc.sync.dma_start(out=t[:], in_=xqT[dc * P:(dc + 1) * P, :])
                    xq_sb.append(t)
                for t in range(HC):
                    pt = qps.tile([P, QN], f32, tag="qps")
                    for dc in range(DC):
                        nc.tensor.matmul(pt[:], wq_sb[dc][:, t * P:(t + 1) * P],
                                         xq_sb[dc][:],
                                         start=(dc == 0), stop=(dc == DC - 1))
                    q_t = qtp.tile([P, QN], f32r, tag="qt")
                    nc.vector.tensor_scalar(q_t[:], pt[:], bq_sb[:, t:t + 1],
                                            1.0 / np.sqrt(KD), OP.add, OP.mult)
                    qt_sb.append(q_t)

            ctxn = []
            for g in range(NG):
                with tc.tile_pool(name="ktg", bufs=NKT) as ktpool:
                    # ---- K^T group [GW, S] ----
                    kt_sb = []
                    with tc.tile_pool(name="wk", bufs=DC) as wkp, \
                         tc.tile_pool(name="kps", bufs=1, space="PSUM") as kps:
                        wk_sb = []
                        for dc in range(DC):
                            t = wkp.tile([P, GW], f32r, tag="wk")
                            nc.sync.dma_start(
                                out=t[:],
                                in_=wk[dc * P:(dc + 1) * P, g * GW:(g + 1) * GW])
                            wk_sb.append(t)
                        for t in range(NKT):
                            pt = kps.tile([P, S_], f32, tag="kps")
                            for dc in range(DC):
                                for s5 in range(NS5):
                                    nc.tensor.matmul(
                                        pt[:, s5 * S5:(s5 + 1) * S5],
                                        wk_sb[dc][:, t * P:(t + 1) * P],
                                        xt_sb[dc][:, s5 * S5:(s5 + 1) * S5],
                                        start=(dc == 0), stop=(dc == DC - 1))
                            kt_t = ktpool.tile([P, S_], f32r, tag="kt")
                            nc.vector.tensor_scalar(
                                kt_t[:], pt[:],
                                bk_sb[:, (g * NKT + t):(g * NKT + t) + 1],
                                None, OP.add)
                            kt_sb.append(kt_t)

                    # ---- attention, V produced just-in-time per s-chunk ----
                    with tc.tile_pool(name="wv", bufs=DC) as wvp, \
                         tc.tile_pool(name="vaug", bufs=3) as vaugp, \
                         tc.tile_pool(name="exp", bufs=3) as epool, \
                         tc.tile_pool(name="rdp", bufs=2) as rdpool, \
                         tc.tile_pool(name="rbp", bufs=2) as rbpool, \
                         tc.tile_pool(name="vps", bufs=2, space="PSUM") as vps, \
                         tc.tile_pool(name="sps", bufs=2, space="PSUM") as sps, \
                         tc.tile_pool(name="cps", bufs=GH, space="PSUM") as cps:
                        wv_sb = []
                        for dc in range(DC):
                            t = wvp.tile([P, GW], f32r, tag="wv")
                            nc.sync.dma_start(
                                out=t[:],
                                in_=wv[dc * P:(dc + 1) * P, g * GW:(g + 1) * GW])
                            wv_sb.append(t)
                        pc = [cps.tile([P, QN], f32, tag="cps",
                                       name=f"pc{g}_{i}")
                              for i in range(GH)]
                        for i in range(GH):
                            # ACT write absorbs the slot-release wait so the
                            # first ctx matmul only waits on ACT
                            nc.scalar.activation(pc[i][0:1, 0:2],
                                                 bq_sb[0:1, 0:2], AF.Copy,
                                                 scale=0.0)
                        for sc in range(NSC):
                            pv = vps.tile([P, GW], f32, tag="vps")
                            for dc in range(DC):
                                nc.tensor.matmul(
                                    pv[:], xt_sb[dc][:, sc * P:(sc + 1) * P],
                                    wv_sb[dc][:],
                                    start=(dc == 0), stop=False)
                            # + bv broadcast via K=1 matmul (keeps va ACT-only)
                            nc.tensor.matmul(
                                pv[:], ones_r[0:1, 0:P],
                                bv_sb[0:1, g * GW:(g + 1) * GW],
                                start=False, stop=True)
                            va = vaugp.tile([P, GH * 65], f32r, tag="va")
                            vav = va[:].rearrange("p (h k) -> p h k", k=65)
                            nc.scalar.copy(
                                vav[:, :, 0:64],
                                pv[:].rearrange("p (h k) -> p h k", k=64))
                            nc.scalar.activation(
                                vav[:, :, 64:65],
                                pv[:, 0:GH].rearrange("p (h o) -> p h o", o=1),
                                AF.Copy, bias=1.0, scale=0.0)
                            for hl in range(GH):
                                h = g * GH + hl
                                po = (hl * KD) % P
                                qtile = qt_sb[(h * KD) // P]
                                qpo = (h * KD) % P
                                ps = sps.tile([P, QN], f32, tag="sps")
                                nc.tensor.matmul(
                                    ps[:],
                                    kt_sb[(hl * KD) // P][po:po + KD,
                                                          sc * P:(sc + 1) * P],
                                    qtile[qpo:qpo + KD, :],
                                    start=True, stop=True)
                                et = epool.tile([P, QN], f32r, tag="exp")
                                nc.scalar.activation(et[:], ps[:], AF.Exp)
                                nc.tensor.matmul(
                                    pc[hl][0:65, :],
                                    va[:, hl * 65:(hl + 1) * 65],
                                    et[:],
                                    start=(sc == 0), stop=(sc == NSC - 1))
                        # normalize: ctxT[0:64] * (1/denom row 64) bcast
                        for hl in range(GH):
                            rd = rdpool.tile([65, QN], f32, tag="rd")
                            nc.vector.reciprocal(rd[64:65, :], pc[hl][64:65, :])
                            prb = sps.tile([64, QN], f32, tag="sps")
                            nc.tensor.matmul(prb[:], ones[64:65, 0:64],
                                             rd[64:65, :], start=True,
                                             stop=True)
                            rb = rbpool.tile([64, QN], f32r, tag="rb")
                            nc.vector.tensor_copy(rb[:], prb[:])
                            cn = cnp.tile([64, QN], f32r, tag="cn")
                            nc.vector.tensor_tensor(cn[:], pc[hl][0:64, :],
                                                    rb[:], OP.mult)
                            ctxn.append(cn)

            # release x^T and Q^T before the out-projection phase
            qt_scope.__exit__(None, None, None)
            xtq_scope.__exit__(None, None, None)

            # ---- output projection + residual + LayerNorm ----
            with tc.tile_pool(name="wo", bufs=H_) as wop, \
                 tc.tile_pool(name="lnB", bufs=1) as lbp, \
                 tc.tile_pool(name="xq2", bufs=2) as xqp2, \
                 tc.tile_pool(name="ln", bufs=2) as lnp, \
                 tc.tile_pool(name="st", bufs=8) as stp, \
                 tc.tile_pool(name="ops", bufs=2, space="PSUM") as ops:
                wo_sb = []
                for h in range(H_):
                    t = wop.tile([KD, D_], f32r, tag="wo")
                    nc.sync.dma_start(out=t[:], in_=wo[h * KD:(h + 1) * KD, :])
                    wo_sb.append(t)
                boB = lbp.tile([P, D_], f32, tag="boB")
                gmB = lbp.tile([P, D_], f32, tag="gmB")
                btB = lbp.tile([P, D_], f32, tag="btB")
                nc.sync.dma_start(out=boB[:], in_=bop[:].to_broadcast((P, D_)))
                nc.sync.dma_start(out=gmB[:], in_=gmp[:].to_broadcast((P, D_)))
                nc.sync.dma_start(out=btB[:], in_=btp[:].to_broadcast((P, D_)))

                # PE observes the last attn DVE tick once, so the first
                # real out-proj matmul only waits on its wo DMA queue
                obs = ops.tile([1, 2], f32, tag="ops", name="obs")
                nc.tensor.matmul(obs[:], ctxn[H_ - 1][:, 0:1],
                                 ctxn[H_ - 1][:, 0:2], start=True, stop=True)
                for qc in range(NQC):
                    po_ = ops.tile([P, D_], f32, tag="ops")
                    for h in range(H_):
                        for d5 in range(ND5):
                            nc.tensor.matmul(
                                po_[:, d5 * D5:(d5 + 1) * D5],
                                ctxn[h][:, qc * P:(qc + 1) * P],
                                wo_sb[h][:, d5 * D5:(d5 + 1) * D5],
                                start=(h == 0), stop=(h == H_ - 1))
                    xq_t = xqp2.tile([P, D_], f32, tag="xq2")
                    nc.sync.dma_start(out=xq_t[:], in_=xq[qc * P:(qc + 1) * P, :])
                    y = lnp.tile([P, D_], f32, tag="y")
                    nc.vector.tensor_tensor(y[:], po_[:], xq_t[:], OP.add)
                    nc.vector.tensor_tensor(y[:], y[:], boB[:], OP.add)
                    sum_t = stp.tile([P, 1], f32, tag="sum")
                    nc.vector.reduce_sum(out=sum_t[:], in_=y[:], axis=AX)
                    mean_t = stp.tile([P, 1], f32, tag="mean")
                    nc.vector.tensor_scalar_mul(mean_t[:], sum_t[:], 1.0 / D_)
                    cent = lnp.tile([P, D_], f32, tag="cent")
                    nc.vector.tensor_scalar(cent[:], y[:], mean_t[:], None,
                                            OP.subtract)
                    sq = lnp.tile([P, D_], f32, tag="sq")
                    vs = stp.tile([P, 1], f32, tag="vs")
                    nc.scalar.activation(sq[:], cent[:], AF.Square,
                                         accum_out=vs[:])
                    std = stp.tile([P, 1], f32, tag="std")
                    nc.scalar.activation(std[:], vs[:], AF.Sqrt,
                                         bias=eps_t[:], scale=1.0 / D_)
                    rstd = stp.tile([P, 1], f32, tag="rstd")
                    nc.vector.reciprocal(rstd[:], std[:])
                    nrm = lnp.tile([P, D_], f32, tag="nrm")
                    nc.vector.tensor_scalar_mul(nrm[:], cent[:], rstd[:])
                    ot = lnp.tile([P, D_], f32, tag="ot")
                    nc.vector.tensor_tensor(ot[:], nrm[:], gmB[:], OP.mult)
                    nc.vector.tensor_tensor(ot[:], ot[:], btB[:], OP.add)
                    nc.sync.dma_start(out=out[qc * P:(qc + 1) * P, :], in_=ot[:])

    # Post-pass: walrus's per-instruction ISA structs hold only ONE sync
    # wait for compute-engine instructions (S3_LW for matmul, S3D3_TS for
    # tensor_scalar, ...). Move excess waits onto standalone
    # EventSemaphore instructions placed just before on the same engine
    # stream (sequencer executes them in order; semantics unchanged).
    SPLIT = {"InstMatmult", "InstTensorScalarPtr", "InstTensorScalar",
             "InstTensorTensor", "InstReciprocal", "InstActivation",
             "InstTensorReduce", "InstTensorCopy", "InstMemSet",
             "InstCopy", "InstDMACopy", "InstDMATranspose", "InstDrain"}
    evt_n = 0
    for f in nc.m.functions:
        for bb in f.blocks:
            need = any(
                type(i).__name__ in SPLIT and i.sync_info is not None
                and len(i.sync_info.on_wait) > 1 for i in bb.instructions)
            if not need:
                continue
            newl = []
            for ins in bb.instructions:
                si = ins.sync_info
                if (type(ins).__name__ in SPLIT and si is not None
                        and len(si.on_wait) > 1):
                    extra = list(si.on_wait[:-1])
                    for j in range(0, len(extra), 2):  # evt-sem holds <=2
                        evt_n += 1
                        evt = mybir.InstEventSemaphore(name=f"mmwait_{evt_n}")
                        evt.engine = ins.engine
                        evt.sync_info = mybir.SyncInfo(
                            on_wait=extra[j:j + 2], on_update=[])
                        newl.append(evt)
                    ins.sync_info = mybir.SyncInfo(
                        on_wait=[si.on_wait[-1]],
                        on_update=list(si.on_update))
                newl.append(ins)
            bb.instructions = newl
    return nc


def get_nc(D_=D, S_=S, SB_=SB, H_=H):
    key = (D_, S_, SB_, H_)
    if key not in _cache:
        _cache[key] = _build(D_, S_, SB_, H_)
    return _cache[key]


def make_in_maps(inputs, D_=D, S_=S, SB_=SB, H_=H, n_cores=8):
    """Shard full inputs into per-core input maps (host-side layout prep)."""
    HK_ = H_ * KD
    HC = HK_ // P
    nb = inputs["x"].shape[0]
    nq = n_cores // nb
    f = np.float32
    wq_ = np.ascontiguousarray(inputs["wq"].reshape(D_, HK_), f)
    wk_ = np.ascontiguousarray(inputs["wk"].reshape(D_, HK_), f)
    wv_ = np.ascontiguousarray(inputs["wv"].reshape(D_, HK_), f)
    wo_ = np.ascontiguousarray(inputs["wo"].reshape(HK_, D_), f)
    bqT = np.ascontiguousarray(np.asarray(inputs["bq"], f).reshape(HC, P).T)
    bkT = np.ascontiguousarray(np.asarray(inputs["bk"], f).reshape(HC, P).T)
    bv_row = np.asarray(inputs["bv"], f).reshape(1, HK_)
    bo_row = np.asarray(inputs["bo"], f).reshape(1, D_)
    gm_row = np.asarray(inputs["gamma"], f).reshape(1, D_)
    bt_row = np.asarray(inputs["beta"], f).reshape(1, D_)
    maps = []
    for c in range(n_cores):
        b, qb = c // nq, c % nq
        xb = np.asarray(inputs["x"][b], f)
        xTb = np.ascontiguousarray(xb.T)
        maps.append(dict(
            xT=xTb,
            xqT=np.ascontiguousarray(xTb[:, qb * SB_:(qb + 1) * SB_]),
            xq=np.ascontiguousarray(xb[qb * SB_:(qb + 1) * SB_]),
            wq=wq_, wk=wk_, wv=wv_, wo=wo_,
            bqT=bqT, bkT=bkT, bv_row=bv_row, bo_row=bo_row,
            gamma_row=gm_row, beta_row=bt_row,
        ))
    return maps


def kernel(**inputs):
    from concourse.bass_utils import run_bass_kernel_spmd
    nc = get_nc()
    maps = make_in_maps(inputs)
    res = run_bass_kernel_spmd(nc, maps, list(range(8)))
    x = inputs["x"]
    outp = np.empty((B, S, D), np.float32)
    nq = 8 // B
    for c in range(8):
        b, qb = c // nq, c % nq
        outp[b, qb * SB:(qb + 1) * SB] = res.results[c]["out"]
    return outp



# revision 2
# speedup vs baseline: 1.0231x; 1.0231x over previous
"""Trainium2 Bass kernel v2 for nn_BaseAttention (B=2,S=2048,D=1024,H=16,K=64).

Sharding: 8 cores = (batch b in {0,1}) x (block qb in {0..3}, 512 rows).
Unlike v1 (which computed K/V for the FULL sequence on every core), each
core projects Q, K, V only for its OWN 512-row block, then K^T and an
augmented V (65th all-ones column per head, for the softmax denominator)
are AllGather'd in bf16 across the 4 cores of each batch:
  replica_groups = [[0,1,2,3],[4,5,6,7]]
The gathers are split per head-group (4 K gathers + 4 V gathers) so
attention on head-group g can start as soon as AG_K[g] lands, overlapping
the remaining gathers with compute.

All tensor data is bf16 (inputs rounded host-side); matmul accumulation
is f32 in PSUM; LayerNorm stats in f32. rel-err budget is 2e-2, bf16
rounding lands ~5e-3.

Per-core phases:
  1. K proj (own block) -> bf16 -> DRAM bounce -> AG_K[g] per head-group
     Q proj tiles 0-1, V proj (own block) -> augmented-V bounce -> AG_V[g]
     Q proj tiles 2-7
  2. per head-group g: read back gathered K^T/V-aug, scores -> exp(bf16)
     -> ctx accumulate (65-row PSUM: row 64 = softmax denominator)
  3. normalize: ctx/denom via reciprocal row + ones-matmul broadcast
     (emitted interleaved into the NEXT group's score loop to keep the
     in-order PE queue from stalling on the DVE normalize chain)
  4. output projection + residual + LayerNorm on own 512 rows.
"""

import sys
import numpy as np

B, S, D, H, KD = 2, 2048, 1024, 16, 64
SB = S // 4          # per-core block of queries / keys
HK = H * KD
P = 128

if "/opt/trn_rl_repo" not in sys.path:
    sys.path.insert(0, "/opt/trn_rl_repo")

_cache = {}


def _build():
    import concourse.bass as bass
    import concourse.mybir as mybir
    from concourse.tile import TileContext

    dt = mybir.dt
    f32, f32r, bf16 = dt.float32, dt.float32r, dt.bfloat16
    AF = mybir.ActivationFunctionType
    OP = mybir.AluOpType
    AX = mybir.AxisListType.X

    DC = D // P              # 8 d-chunks of 128
    HC = HK // P             # 8 hk tiles
    NG = 4                   # head groups (4 heads each)
    GH = H // NG             # 4
    NSC = S // P             # 16 global s-chunks
    NQC = SB // P            # 4 q-chunks
    NR = 4                   # ranks per gather group
    W65 = GH * 65            # augmented-V group width = 260

    nc = bass.Bass(num_devices=8)
    xqT = nc.declare_dram_parameter("xqT", [D, SB], bf16, isOutput=False)
    xq = nc.declare_dram_parameter("xq", [SB, D], f32, isOutput=False)
    wq = nc.declare_dram_parameter("wq", [D, HK], bf16, isOutput=False)
    wk = nc.declare_dram_parameter("wk", [D, HK], bf16, isOutput=False)
    wv = nc.declare_dram_parameter("wv", [D, HK], bf16, isOutput=False)
    wo = nc.declare_dram_parameter("wo", [HK, D], bf16, isOutput=False)
    bqp = nc.declare_dram_parameter("bqT", [P, HC], f32, isOutput=False)
    bkp = nc.declare_dram_parameter("bkT", [P, HC], f32, isOutput=False)
    bvp = nc.declare_dram_parameter("bv_row", [1, HK], f32, isOutput=False)
    bop = nc.declare_dram_parameter("bo_row", [1, D], f32, isOutput=False)
    gmp = nc.declare_dram_parameter("gamma_row", [1, D], f32, isOutput=False)
    btp = nc.declare_dram_parameter("beta_row", [1, D], f32, isOutput=False)
    out = nc.declare_dram_parameter("out", [SB, D], f32, isOutput=True)

    groups = [[0, 1, 2, 3], [4, 5, 6, 7]]

    with TileContext(nc) as tc:
        with tc.tile_pool(name="const", bufs=1) as cpool, \
             tc.tile_pool(name="dramb", bufs=1, space="DRAM") as dbp, \
             tc.tile_pool(name="ctxn", bufs=H) as cnp, \
             tc.tile_pool(name="ctxu", bufs=H) as cup:

            ones = cpool.tile([128, P], f32, tag="ones")
            nc.vector.memset(ones[:], 1.0)
            eps_t = cpool.tile([P, 1], f32, tag="eps")
            nc.vector.memset(eps_t[:], 1e-6)

            # Dummy warm-up collective issued immediately: absorbs the
            # one-time CC-stream entry barrier (~75us) while the projection
            # phase runs, so the real gathers start with ~1us trigger delay.
            dmy_i = dbp.tile([1, 16], f32, tag="dmy_i")
            dmy_o = dbp.tile([4, 16], f32, tag="dmy_o")
            dmy_s = cpool.tile([1, 16], f32, tag="dmy_s")
            nc.vector.memset(dmy_s[:], 0.0)
            nc.sync.dma_start(out=dmy_i[:], in_=dmy_s[:])
            nc.gpsimd.collective_compute(
                "AllGather", OP.bypass, [[0, 1, 2, 3], [4, 5, 6, 7]],
                ins=[dmy_i.opt()], outs=[dmy_o.opt()])

            bq_sb = cpool.tile([P, HC], f32, tag="bq")
            nc.sync.dma_start(out=bq_sb[:], in_=bqp[:])
            bk_sb = cpool.tile([P, HC], f32, tag="bk")
            nc.sync.dma_start(out=bk_sb[:], in_=bkp[:])
            bvB = cpool.tile([P, HK], f32, tag="bvB")
            nc.sync.dma_start(out=bvB[:], in_=bvp[:].to_broadcast((P, HK)))

            # DRAM bounce buffers for the gathers (one pair per head-group)
            bk_in = [dbp.tile([2 * P, SB], bf16, tag=f"bki{g}", name=f"bki{g}")
                     for g in range(NG)]
            bk_out = [dbp.tile([NR * 2 * P, SB], bf16, tag=f"bko{g}",
                               name=f"bko{g}") for g in range(NG)]
            bv_in = [dbp.tile([SB, W65], bf16, tag=f"bvi{g}", name=f"bvi{g}")
                     for g in range(NG)]
            bv_out = [dbp.tile([NR * SB, W65], bf16, tag=f"bvo{g}",
                               name=f"bvo{g}") for g in range(NG)]

            # ---- load x^T (own block) + all projection weights ----
            # (QT pool entered first: it outlives the others, LIFO closes)
            qt_scope = tc.tile_pool(name="QT", bufs=HC)
            qtp = qt_scope.__enter__()
            xtq_scope = tc.tile_pool(name="xT", bufs=DC)
            xtp = xtq_scope.__enter__()
            wk_scope = tc.tile_pool(name="wk", bufs=DC)
            wkp = wk_scope.__enter__()
            wv_scope = tc.tile_pool(name="wv", bufs=DC)
            wvp = wv_scope.__enter__()
            wq_scope = tc.tile_pool(name="wq", bufs=DC)
            wqp = wq_scope.__enter__()

            xt_sb = []
            for dc in range(DC):
                t = xtp.tile([P, SB], bf16, tag="xt")
                nc.sync.dma_start(out=t[:], in_=xqT[dc * P:(dc + 1) * P, :])
                xt_sb.append(t)
            wk_sb = []
            for dc in range(DC):
                t = wkp.tile([P, HK], bf16, tag="wk")
                nc.sync.dma_start(out=t[:], in_=wk[dc * P:(dc + 1) * P, :])
                wk_sb.append(t)
            wv_sb = []
            for dc in range(DC):
                t = wvp.tile([P, HK], bf16, tag="wv")
                nc.sync.dma_start(out=t[:], in_=wv[dc * P:(dc + 1) * P, :])
                wv_sb.append(t)
            wq_sb = []
            for dc in range(DC):
                t = wqp.tile([P, HK], bf16, tag="wq")
                nc.sync.dma_start(out=t[:], in_=wq[dc * P:(dc + 1) * P, :])
                wq_sb.append(t)

            qt_sb = [None] * HC

            def q_proj(t, qps):
                pt = qps.tile([P, SB], f32, tag="qps")
                for dc in range(DC):
                    nc.tensor.matmul(pt[:], wq_sb[dc][:, t * P:(t + 1) * P],
                                     xt_sb[dc][:],
                                     start=(dc == 0), stop=(dc == DC - 1))
                q_t = qtp.tile([P, SB], bf16, tag="qt")
                nc.vector.tensor_scalar(q_t[:], pt[:], bq_sb[:, t:t + 1],
                                        1.0 / np.sqrt(KD), OP.add, OP.mult)
                qt_sb[t] = q_t

            # ---- K/V/Q projections (own block), bf16, gathers interleaved
            # in need-order on the single CC stream: K0, V0, V1, K1-3, V2-3
            with tc.tile_pool(name="kl", bufs=3) as klp, \
                 tc.tile_pool(name="vl", bufs=3) as vlp, \
                 tc.tile_pool(name="kps", bufs=2, space="PSUM") as kps, \
                 tc.tile_pool(name="vps", bufs=2, space="PSUM") as vps, \
                 tc.tile_pool(name="qps", bufs=2, space="PSUM") as qps:

                def k_proj(t):
                    pt = kps.tile([P, SB], f32, tag="kps")
                    for dc in range(DC):
                        nc.tensor.matmul(pt[:], wk_sb[dc][:, t * P:(t + 1) * P],
                                         xt_sb[dc][:],
                                         start=(dc == 0), stop=(dc == DC - 1))
                    kt_t = klp.tile([P, SB], bf16, tag="kl")
                    nc.vector.tensor_scalar(kt_t[:], pt[:], bk_sb[:, t:t + 1],
                                            None, OP.add)
                    g, tl = t // 2, t % 2
                    nc.sync.dma_start(out=bk_in[g][tl * P:(tl + 1) * P, :],
                                      in_=kt_t[:])

                def ag_k(g):
                    nc.gpsimd.collective_compute(
                        "AllGather", OP.bypass, groups,
                        ins=[bk_in[g].opt()], outs=[bk_out[g].opt()])

                def v_proj_half(half):
                    for sc in range(NQC):
                        pv = vps.tile([P, HK // 2], f32, tag="vps")
                        for dc in range(DC):
                            nc.tensor.matmul(
                                pv[:],
                                xt_sb[dc][:, sc * P:(sc + 1) * P],
                                wv_sb[dc][:, half * 512:(half + 1) * 512],
                                start=(dc == 0), stop=(dc == DC - 1))
                        for gl in range(2):
                            g, co = 2 * half + gl, gl * 256
                            va = vlp.tile([P, W65], bf16, tag="vl")
                            vav = va[:].rearrange("p (h k) -> p h k", k=65)
                            nc.vector.tensor_tensor(
                                vav[:, :, 0:64],
                                pv[:, co:co + 256].rearrange(
                                    "p (h k) -> p h k", k=64),
                                bvB[:, g * 256:(g + 1) * 256].rearrange(
                                    "p (h k) -> p h k", k=64),
                                OP.add)
                            nc.vector.memset(vav[:, :, 64:65], 1.0)
                            nc.sync.dma_start(
                                out=bv_in[g][sc * P:(sc + 1) * P, :], in_=va[:])

                def ag_v(g):
                    nc.gpsimd.collective_compute(
                        "AllGather", OP.bypass, groups,
                        ins=[bv_in[g].opt()], outs=[bv_out[g].opt()])

                k_proj(0)
                k_proj(1)
                ag_k(0)
                q_proj(0, qps)
                q_proj(1, qps)
                v_proj_half(0)
                ag_v(0)
                ag_v(1)
                for t in range(2, HC):
                    k_proj(t)
                    if t % 2 == 1:
                        ag_k(t // 2)
                v_proj_half(1)
                ag_v(2)
                ag_v(3)
                for t in range(2, HC):
                    q_proj(t, qps)

            # weights and x^T are no longer needed; free SBUF before the
            # attention phase (gathered K/V + Q + ctx stay resident)
            wq_scope.__exit__(None, None, None)
            wv_scope.__exit__(None, None, None)
            wk_scope.__exit__(None, None, None)
            xtq_scope.__exit__(None, None, None)

            # ---- attention per head-group over gathered K/V ----
            ctxu = [None] * H   # unnormalized ctx+denom [65, SB] bf16
            ctxn = [None] * H   # normalized ctx [64, SB] bf16
            pend = []           # deferred normalize steps for previous group

            with tc.tile_pool(name="ktg", bufs=16) as ktgp, \
                 tc.tile_pool(name="vag", bufs=32) as vagp, \
                 tc.tile_pool(name="exp", bufs=3) as epool, \
                 tc.tile_pool(name="rdp", bufs=2) as rdpool, \
                 tc.tile_pool(name="rbp", bufs=4) as rbpool, \
                 tc.tile_pool(name="sps", bufs=2, space="PSUM") as sps, \
                 tc.tile_pool(name="prb", bufs=2, space="PSUM") as prp, \
                 tc.tile_pool(name="cps", bufs=GH, space="PSUM") as cps:

                def normalize_steps(g):
                    """Return list of thunks: drain pc->sbuf, then recip+
                    broadcast+mult, to be interleaved into the next group."""
                    steps = []
                    pcs = pc_of[g]

                    def drain(hl=0):
                        h = g * GH + hl
                        cu = cup.tile([65, SB], bf16, tag="cu", name=f"cu{h}")
                        nc.vector.tensor_copy(cu[:], pcs[hl][0:65, :])
                        ctxu[h] = cu
                    for hl in range(GH):
                        steps.append(lambda hl=hl: drain(hl))

                    def bcast(hl=0):
                        h = g * GH + hl
                        rd = rdpool.tile([65, SB], f32, tag="rd")
                        nc.vector.reciprocal(rd[64:65, :], ctxu[h][64:65, :])
                        pr = prp.tile([64, SB], f32, tag="prb")
                        nc.tensor.matmul(
                            pr[:], ones[64:65, 0:64], rd[64:65, :],
                            start=True, stop=True)
                        rb = rbpool.tile([64, SB], bf16, tag="rb")
                        nc.vector.tensor_copy(rb[:], pr[:])
                        # normalized ctx lands in head-PAIR tiles [128, SB]
                        # so the out-projection contracts 128 rows (2 heads)
                        # per matmul instead of 64
                        if h % 2 == 0:
                            ctxn[h // 2] = cnp.tile([2 * KD, SB], bf16,
                                                    tag="cn", name=f"cn{h}")
                        ro = (h % 2) * KD
                        nc.vector.tensor_tensor(
                            ctxn[h // 2][ro:ro + KD, :], ctxu[h][0:64, :],
                            rb[:], OP.mult)
                    for hl in range(GH):
                        steps.append(lambda hl=hl: bcast(hl))
                    return steps

                pc_of = {}
                for g in range(NG):
                    # read back gathered K^T and V-aug for this group
                    ktg = []
                    for r in range(NR):
                        for tl in range(2):
                            kt = ktgp.tile([P, SB], bf16, tag="ktg")
                            ro = (r * 2 + tl) * P
                            nc.sync.dma_start(out=kt[:],
                                              in_=bk_out[g][ro:ro + P, :])
                            ktg.append(kt)
                    vag = []
                    for scg in range(NSC):
                        va = vagp.tile([P, W65], bf16, tag="vag")
                        nc.sync.dma_start(
                            out=va[:],
                            in_=bv_out[g][scg * P:(scg + 1) * P, :])
                        vag.append(va)

                    pc = [cps.tile([P, SB], f32, tag="cps", name=f"pc{g}_{i}")
                          for i in range(GH)]
                    pc_of[g] = pc
                    for sc in range(NSC):
                        r, scl = sc // 4, sc % 4
                        for hl in range(GH):
                            h = g * GH + hl
                            po = (hl % 2) * 64
                            ktile = ktg[r * 2 + hl // 2]
                            qtile = qt_sb[h // 2]
                            qpo = (h % 2) * 64
                            ps = sps.tile([P, SB], f32, tag="sps")
                            nc.tensor.matmul(
                                ps[:],
                                ktile[po:po + 64, scl * P:(scl + 1) * P],
                                qtile[qpo:qpo + 64, :],
                                start=True, stop=True)
                            et = epool.tile([P, SB], bf16, tag="exp")
                            nc.scalar.activation(et[:], ps[:], AF.Exp)
                            nc.tensor.matmul(
                                pc[hl][0:65, :],
                                vag[sc][:, hl * 65:(hl + 1) * 65],
                                et[:],
                                start=(sc == 0), stop=(sc == NSC - 1))
                        # interleave previous group's normalize steps
                        if pend and sc >= 1:
                            pend.pop(0)()
                    while pend:
                        pend.pop(0)()
                    pend = normalize_steps(g)
                while pend:
                    pend.pop(0)()

            # release Q^T before the out-projection phase
            qt_scope.__exit__(None, None, None)

            # ---- output projection + residual + LayerNorm ----
            with tc.tile_pool(name="wo", bufs=H) as wop, \
                 tc.tile_pool(name="lnB", bufs=1) as lbp, \
                 tc.tile_pool(name="xq2", bufs=2) as xqp2, \
                 tc.tile_pool(name="ln", bufs=2) as lnp, \
                 tc.tile_pool(name="st", bufs=8) as stp, \
                 tc.tile_pool(name="ops", bufs=2, space="PSUM") as ops:
                wo_sb = []
                for hp in range(H // 2):
                    w = wop.tile([P, D], bf16, tag="wo")
                    nc.sync.dma_start(out=w[:], in_=wo[hp * P:(hp + 1) * P, :])
                    wo_sb.append(w)
                boB = lbp.tile([P, D], f32, tag="boB")
                gmB = lbp.tile([P, D], f32, tag="gmB")
                btB = lbp.tile([P, D], f32, tag="btB")
                nc.sync.dma_start(out=boB[:], in_=bop[:].to_broadcast((P, D)))
                nc.sync.dma_start(out=gmB[:], in_=gmp[:].to_broadcast((P, D)))
                nc.sync.dma_start(out=btB[:], in_=btp[:].to_broadcast((P, D)))

                for qc in range(NQC):
                    po_ = ops.tile([P, D], f32, tag="ops")
                    for d5 in range(2):
                        for hp in range(H // 2):
                            nc.tensor.matmul(
                                po_[:, d5 * 512:(d5 + 1) * 512],
                                ctxn[hp][:, qc * P:(qc + 1) * P],
                                wo_sb[hp][:, d5 * 512:(d5 + 1) * 512],
                                start=(hp == 0), stop=(hp == H // 2 - 1))
                    xq_t = xqp2.tile([P, D], f32, tag="xq2")
                    nc.sync.dma_start(out=xq_t[:], in_=xq[qc * P:(qc + 1) * P, :])
                    y = lnp.tile([P, D], f32, tag="y")
                    nc.vector.tensor_tensor(y[:], po_[:], xq_t[:], OP.add)
                    nc.vector.tensor_tensor(y[:], y[:], boB[:], OP.add)
                    sum_t = stp.tile([P, 1], f32, tag="sum")
                    nc.vector.reduce_sum(out=sum_t[:], in_=y[:], axis=AX)
                    mean_t = stp.tile([P, 1], f32, tag="mean")
                    nc.vector.tensor_scalar_mul(mean_t[:], sum_t[:], 1.0 / D)
                    cent = lnp.tile([P, D], f32, tag="cent")
                    nc.vector.tensor_scalar(cent[:], y[:], mean_t[:], None,
                                            OP.subtract)
                    sq = lnp.tile([P, D], f32, tag="sq")
                    vs = stp.tile([P, 1], f32, tag="vs")
                    nc.scalar.activation(sq[:], cent[:], AF.Square,
                                         accum_out=vs[:])
                    std = stp.tile([P, 1], f32, tag="std")
                    nc.scalar.activation(std[:], vs[:], AF.Sqrt,
                                         bias=eps_t[:], scale=1.0 / D)
                    rstd = stp.tile([P, 1], f32, tag="rstd")
                    nc.vector.reciprocal(rstd[:], std[:])
                    nrm = lnp.tile([P, D], f32, tag="nrm")
                    nc.vector.tensor_scalar_mul(nrm[:], cent[:], rstd[:])
                    ot = lnp.tile([P, D], f32, tag="ot")
                    nc.vector.tensor_tensor(ot[:], nrm[:], gmB[:], OP.mult)
                    nc.vector.tensor_tensor(ot[:], ot[:], btB[:], OP.add)
                    nc.sync.dma_start(out=out[qc * P:(qc + 1) * P, :], in_=ot[:])

    # Post-pass: walrus's per-instruction ISA structs hold only ONE sync
    # wait for compute-engine instructions. Move excess waits onto
    # standalone EventSemaphore instructions just before, same engine.
    SPLIT = {"InstMatmult", "InstTensorScalarPtr", "InstTensorScalar",
             "InstTensorTensor", "InstReciprocal", "InstActivation",
             "InstTensorReduce", "InstTensorCopy", "InstMemset",
             "InstCopy", "InstDMACopy", "InstDMATranspose", "InstDrain",
             "InstCollectiveCompute", "InstLdweights"}
    evt_n = 0
    for f in nc.m.functions:
        for bb in f.blocks:
            need = any(
                type(i).__name__ in SPLIT and i.sync_info is not None
                and len(i.sync_info.on_wait) > 1 for i in bb.instructions)
            if not need:
                continue
            newl = []
            for ins in bb.instructions:
                si = ins.sync_info
                if (type(ins).__name__ in SPLIT and si is not None
                        and len(si.on_wait) > 1):
                    extra = list(si.on_wait[:-1])
                    for j in range(0, len(extra), 2):  # evt-sem holds <=2
                        evt_n += 1
                        evt = mybir.InstEventSemaphore(name=f"mmwait_{evt_n}")
                        evt.engine = ins.engine
                        evt.sync_info = mybir.SyncInfo(
                            on_wait=extra[j:j + 2], on_update=[])
                        newl.append(evt)
                    ins.sync_info = mybir.SyncInfo(
                        on_wait=[si.on_wait[-1]],
                        on_update=list(si.on_update))
                newl.append(ins)
            bb.instructions = newl
    return nc


def get_nc():
    if "nc" not in _cache:
        _cache["nc"] = _build()
    return _cache["nc"]


def make_in_maps(inputs, n_cores=8):
    import ml_dtypes
    bf = ml_dtypes.bfloat16
    f = np.float32
    HC = HK // P
    wq_ = np.ascontiguousarray(np.asarray(inputs["wq"], f).reshape(D, HK)).astype(bf)
    wk_ = np.ascontiguousarray(np.asarray(inputs["wk"], f).reshape(D, HK)).astype(bf)
    wv_ = np.ascontiguousarray(np.asarray(inputs["wv"], f).reshape(D, HK)).astype(bf)
    wo_ = np.ascontiguousarray(np.asarray(inputs["wo"], f).reshape(HK, D)).astype(bf)
    bqT = np.ascontiguousarray(np.asarray(inputs["bq"], f).reshape(HC, P).T)
    bkT = np.ascontiguousarray(np.asarray(inputs["bk"], f).reshape(HC, P).T)
    bv_row = np.asarray(inputs["bv"], f).reshape(1, HK)
    bo_row = np.asarray(inputs["bo"], f).reshape(1, D)
    gm_row = np.asarray(inputs["gamma"], f).reshape(1, D)
    bt_row = np.asarray(inputs["beta"], f).reshape(1, D)
    nb = inputs["x"].shape[0]
    nq = n_cores // nb
    maps = []
    for c in range(n_cores):
        b, qb = c // nq, c % nq
        xb = np.asarray(inputs["x"][b], f)
        xTb = np.ascontiguousarray(xb.T[:, qb * SB:(qb + 1) * SB])
        maps.append(dict(
            xqT=xTb.astype(bf),
            xq=np.ascontiguousarray(xb[qb * SB:(qb + 1) * SB]),
            wq=wq_, wk=wk_, wv=wv_, wo=wo_,
            bqT=bqT, bkT=bkT, bv_row=bv_row, bo_row=bo_row,
            gamma_row=gm_row, beta_row=bt_row,
        ))
    return maps


def kernel(**inputs):
    from concourse.bass_utils import run_bass_kernel_spmd
    nc = get_nc()
    maps = make_in_maps(inputs)
    res = run_bass_kernel_spmd(nc, maps, list(range(8)))
    outp = np.empty((B, S, D), np.float32)
    nq = 8 // B
    for c in range(8):
        b, qb = c // nq, c % nq
        outp[b, qb * SB:(qb + 1) * SB] = res.results[c]["out"]
    return outp
